# revision 9
# baseline (speedup 1.0000x reference)
"""CRPS loss kernel for Trainium2, data-parallel over 8 NeuronCores.

Math per (sample n, timestep t): with sorted quantiles q_0..q_10, target y,
padded rows R = [-q_0, q_0..q_10, -q_10] and dz_j = R_{j+2} - R_j:

  CRPS(n,t) = (1/200) * sum_j dz_j * m2_j + y*(1 - o_0 - o_10)
  m2_j = (j-10)^2 if q_j >= y else j^2      (exact small ints in fp16)

This folds the fixed trapezoid weights AND the o-masked central-difference
terms of the CRPS integral into one select between the two integer square
patterns. out[n] = mean_t CRPS(n,t).

Layout: partition = sample (128/block, 4 blocks/core); heavy tensors fp16 in
transposed row layout [p, (i t)] (dense fp16 streams -> DVE 2x; y broadcast
on middle dim). Engine split, all ~9.5us/block:
  SP  : one q DMA per block (+tg, out)
  ACT : f32->f16 transpose-convert, final accumulation (Copy w/ accum)
  DVE : fused compare+select custom op m2, small y-terms
  Pool: dz = shifted subtract, prod = dz*m2 (in place)
"""
import sys

if "/opt/trn_rl_repo" not in sys.path:
    sys.path.insert(0, "/opt/trn_rl_repo")

import numpy as np
import concourse.bass as bass
import concourse.tile as tile
from concourse import bacc, mybir
from concourse import dve_ops as _dve_ops
from concourse.dve_spec import Spec, Src0, Src1, SubIdx, C0, select, sq, lower
from concourse.dve_uop import DveOpSpec
from concourse.bass_utils import run_bass_kernel_spmd
from concourse.alu_op_type import AluOpType

N_CORES = 8
N, T, D = 4096, 512, 11
N_LOC = N // N_CORES        # 512 samples per core
P = 128                     # partitions
BLOCKS = N_LOC // P         # 4
FP = mybir.dt.float32
F16 = mybir.dt.float16
ACT = mybir.ActivationFunctionType


def _register_crps_op():
    """Register the fused compare+select custom DVE op (idempotent)."""
    name = "CRPS_SEL_SQ"
    for op in _dve_ops.OPS:
        if op.name == name:
            return op
    body = select(Src0 >= Src1, sq(SubIdx - C0), sq(SubIdx))

    def _ref(in0, in1, c0, c1, c2):
        x = np.asarray(in0, np.float32)
        j = np.arange(x.shape[1], dtype=np.float32)[None, :, None]
        c0v = c0 if isinstance(c0, float) else np.asarray(c0, np.float32).reshape(-1, 1, 1)
        yb = np.broadcast_to(np.asarray(in1, np.float32), x.shape)
        return np.where(x >= yb, (j - c0v) ** 2, j ** 2)

    spec = Spec(body=body, reference=_ref)
    row = 1 + len(_dve_ops.OPS)
    _dve_ops._SUB_OPCODE_FOR_NAME[name] = row
    shas = {}
    for ver in ("v3", "v4"):
        s = DveOpSpec(name=name, opcode=row, uops=lower(spec, ver=ver), rd1_en=True)
        shas[ver] = s.sha(ver)
    op = _dve_ops.DveOp(name, spec, subdim=True, uops_sha=shas)
    _dve_ops.OPS.append(op)
    _dve_ops.CUSTOM_DVE_SPECS[name] = spec
    return op


CRPS_SEL_SQ = _register_crps_op()


def build_crps_kernel(tc, out_ap, inp_ap, tgt_ap, pools):
    nc = tc.nc
    qpool, mpool, spool, apool = pools
    inp_r = inp_ap.rearrange("(b p) t i -> b p (t i)", p=P)   # [4, 128, 5632]
    tgt_r = tgt_ap.rearrange("(b p) t -> b p t", p=P)          # [4, 128, 512]

    if True:
        s1a = apool.tile([P, BLOCKS], FP, tag="s1a")    # sum dz*m2
        yta = apool.tile([P, BLOCKS], FP, tag="yta")    # sum y*(o0+o10)
        sya = apool.tile([P, BLOCKS], FP, tag="sya")    # sum y
        for b in range(BLOCKS):
            tg32 = spool.tile([P, T], FP, tag="tg32")
            nc.sync.dma_start(tg32[:], tgt_r[b]).annotate(f"tdma{b}")
            q32 = qpool.tile([P, T * D], FP, tag="q32")
            nc.sync.dma_start(q32[:], inp_r[b]).annotate(f"qdma{b}")

            # fp16 transposed padded rows [p, (i t)], quantile i at row i+1
            qpad = mpool.tile([P, 13 * T], F16, tag="qpad")
            mid = qpad[:, T : 12 * T].rearrange("p (i t) -> p t i", t=T)
            nc.scalar.copy(mid, q32[:].rearrange("p (t i) -> p t i", i=D)).annotate(f"conv{b}")
            y16 = spool.tile([P, T], F16, tag="y16")
            nc.vector.tensor_copy(y16[:], tg32[:]).annotate(f"y16_{b}")
            nc.vector.tensor_scalar_mul(qpad[:, 0:T], qpad[:, T : 2 * T], -1.0).annotate(f"pad0_{b}")
            nc.vector.tensor_scalar_mul(qpad[:, 12 * T :], qpad[:, 11 * T : 12 * T], -1.0).annotate(f"pad1_{b}")

            q3 = qpad[:, T : 12 * T].rearrange("p (i t) -> p i t", t=T)
            yb = y16[:].unsqueeze(1).broadcast_to([P, D, T])

            # m2 = select(q >= y, (j-10)^2, j^2)  -- fused custom DVE op
            m2 = mpool.tile([P, D * T], F16, tag="m2")
            nc.vector._custom_dve(
                CRPS_SEL_SQ,
                out=m2[:].rearrange("p (i t) -> p i t", t=T),
                in0=q3, in1=yb, s0=10.0,
            ).annotate(f"m2_{b}")

            # dz = R_{j+2} - R_j on Pool; prod = dz*m2 in place on Pool
            dz = mpool.tile([P, D * T], F16, tag="dz")
            nc.gpsimd.tensor_tensor(
                dz[:], qpad[:, 2 * T :], qpad[:, : 11 * T], AluOpType.subtract
            ).annotate(f"dz{b}")
            nc.gpsimd.tensor_tensor(dz[:], dz[:], m2[:], AluOpType.mult).annotate(f"prod{b}")

            # S = sum prod  (ACT accumulate; scratch out over dead m2)
            nc.scalar.activation(
                m2[:], dz[:], ACT.Copy, accum_out=s1a[:, b : b + 1]
            ).annotate(f"sacc{b}")

            # y-correction: o edge rows (quantiles 0 and 10) vs y
            o2 = spool.tile([P, 2 * T], F16, tag="o2")
            qe = bass.AP(qpad.tensor, qpad.offset + T, [qpad.ap[0], [10 * T, 2], [1, T]])
            nc.vector.tensor_tensor(
                o2[:].rearrange("p (i t) -> p i t", t=T), qe,
                y16[:].unsqueeze(1).broadcast_to([P, 2, T]), AluOpType.is_ge,
            ).annotate(f"oedge{b}")
            osum = spool.tile([P, T], F16, tag="osum")
            nc.vector.tensor_tensor(
                osum[:], o2[:, 0:T], o2[:, T : 2 * T], AluOpType.add
            ).annotate(f"osum{b}")
            nc.vector.scalar_tensor_tensor(
                osum[:], y16[:], 1.0, osum[:], AluOpType.mult, AluOpType.mult,
                accum_out=yta[:, b : b + 1],
            ).annotate(f"yterm{b}")
            # sum_t y (f32)
            nc.vector.tensor_reduce(
                sya[:, b : b + 1], tg32[:], mybir.AxisListType.X, AluOpType.add
            ).annotate(f"sumy{b}")
        # out = (S/200 + sum_y - yterm)/T for all blocks
        r1 = apool.tile([P, BLOCKS], FP, tag="r1")
        nc.vector.tensor_tensor(r1[:], sya[:], yta[:], AluOpType.subtract)
        r2 = apool.tile([P, BLOCKS], FP, tag="r2")
        nc.vector.scalar_tensor_tensor(
            r2[:], s1a[:], 1.0 / 200.0, r1[:], AluOpType.mult, AluOpType.add
        )
        r3 = apool.tile([P, BLOCKS], FP, tag="r3")
        nc.vector.tensor_scalar_mul(r3[:], r2[:], 1.0 / T)
        nc.sync.dma_start(out_ap.rearrange("(b p) -> p b", p=P), r3[:])


def _build_nc(repeat=1):
    nc = bacc.Bacc("TRN2", target_bir_lowering=False, debug=False,
                   num_devices=N_CORES)
    inp = nc.dram_tensor("inp", [N_LOC, T, D], FP, kind="ExternalInput").ap()
    tgt = nc.dram_tensor("target", [N_LOC, T], FP, kind="ExternalInput").ap()
    out = nc.dram_tensor("out", [N_LOC], FP, kind="ExternalOutput").ap()
    unroll = next(u for u in (8, 4, 2, 1) if repeat % u == 0)
    with tile.TileContext(nc) as tc:
        with (
            tc.tile_pool(name="qp", bufs=3) as qpool,
            tc.tile_pool(name="mp", bufs=3) as mpool,
            tc.tile_pool(name="sp", bufs=3) as spool,
            tc.tile_pool(name="acc", bufs=2) as apool,
        ):
            pools = (qpool, mpool, spool, apool)
            if repeat == 1:
                build_crps_kernel(tc, out, inp, tgt, pools)
            else:
                with tc.For_i(0, repeat // unroll, 1, staggered_reset=True):
                    for _ in range(unroll):
                        build_crps_kernel(tc, out, inp, tgt, pools)
    nc.compile()
    return nc


_NC_CACHE = {}


def get_nc(repeat=1):
    if repeat not in _NC_CACHE:
        _NC_CACHE[repeat] = _build_nc(repeat)
    return _NC_CACHE[repeat]


def kernel(inp: np.ndarray, target: np.ndarray) -> np.ndarray:
    inp = np.ascontiguousarray(inp, dtype=np.float32)
    target = np.ascontiguousarray(target, dtype=np.float32)
    nc = get_nc()
    in_maps = [
        {
            "inp": inp[c * N_LOC : (c + 1) * N_LOC],
            "target": target[c * N_LOC : (c + 1) * N_LOC],
        }
        for c in range(N_CORES)
    ]
    res = run_bass_kernel_spmd(nc, in_maps, core_ids=list(range(N_CORES)))
    return np.concatenate([res.results[c]["out"] for c in range(N_CORES)])


# revision 10
# speedup vs baseline: 1.2707x; 1.2707x over previous
"""CRPS loss kernel for Trainium2, data-parallel over 8 NeuronCores.

Math per (sample n, timestep t): with sorted quantiles q_0..q_10, target y,
padded rows R = [-q_0, q_0..q_10, -q_10] and dz_j = R_{j+2} - R_j:

  m2_j  = (j - 10*[q_j >= y])^2          (exact small ints, fp16)
  CRPS  = (1/200)*sum_j dz_j*m2_j - (1/100)*y*(m2_0 - m2_10)

The select between the two trapezoid weight patterns j^2 / (j-10)^2 absorbs
the whole CRPS integral (fixed weights + masked central differences); the
y term falls out of m2's edge columns. out[n] = mean_t CRPS(n,t).

Layout: partition = sample (128/block, 4 blocks/core), natural [p,(t i)]
order end to end - no transpose, no f32->f16 convert pass. Engine split
(HW-measured costs, all close to the ~11.9us/block DMA-bus floor):
  SP  : one 2.9MB q DMA per block (~10.8us, saturates the ~267GB/s bus)
  DVE : fused custom op m2 (7.2us), dz edge cols, 75% of prod, y-term
  Pool: dz central difference (9.2us), 25% of prod
  ACT : the single big accumulation sum(prod) via Copy+accum (6.5us)
"""
import sys

if "/opt/trn_rl_repo" not in sys.path:
    sys.path.insert(0, "/opt/trn_rl_repo")

import numpy as np
import concourse.bass as bass
import concourse.tile as tile
from concourse import bacc, mybir
from concourse import dve_ops as _dve_ops
from concourse.dve_spec import Spec, Src0, Src1, C0, C1, Zero, PageIdx, Idx, sq, lower
from concourse.dve_uop import DveOpSpec
from concourse.bass_utils import run_bass_kernel_spmd
from concourse.alu_op_type import AluOpType

N_CORES = 8
N, T, D = 4096, 512, 11
N_LOC = N // N_CORES        # 512 samples per core
P = 128                     # partitions
BLOCKS = N_LOC // P         # 4
FP = mybir.dt.float32
F16 = mybir.dt.float16
ACT = mybir.ActivationFunctionType

PROD_SPLIT = 8  # quantile columns 0..PROD_SPLIT-1 of prod on DVE, rest on Pool


def _register_crps_op():
    """Fused m2 = sq(in-page-idx - C0*[q >= y]) custom DVE op (idempotent)."""
    name = "CRPS_SEL_SQ2"
    for op in _dve_ops.OPS:
        if op.name == name:
            return op
    jm = Idx - PageIdx(Zero, C1)
    body = sq(jm - C0 * (Src0 >= Src1))

    def _ref(in0, in1, c0, c1, c2):
        x = np.asarray(in0, np.float32)
        j = np.arange(x.shape[2], dtype=np.float32)[None, None, :]
        c0v = c0 if isinstance(c0, float) else np.asarray(c0, np.float32).reshape(-1, 1, 1)
        yb = np.broadcast_to(np.asarray(in1, np.float32), x.shape)
        o = (x >= yb).astype(np.float32)
        return (j - c0v * o) ** 2

    spec = Spec(body=body, reference=_ref)
    row = 1 + len(_dve_ops.OPS)
    _dve_ops._SUB_OPCODE_FOR_NAME[name] = row
    shas = {}
    for ver in ("v3", "v4"):
        s = DveOpSpec(name=name, opcode=row, uops=lower(spec, ver=ver), rd1_en=True)
        shas[ver] = s.sha(ver)
    op = _dve_ops.DveOp(name, spec, subdim=True, uops_sha=shas)
    _dve_ops.OPS.append(op)
    _dve_ops.CUSTOM_DVE_SPECS[name] = spec
    return op


CRPS_SEL_SQ2 = _register_crps_op()


def build_crps_kernel(tc, out_ap, inp_ap, tgt_ap, pools):
    nc = tc.nc
    qpool, mpool, spool, apool = pools
    inp_r = inp_ap.rearrange("(b p) t i -> b p (t i)", p=P)   # [4, 128, 5632]
    tgt_r = tgt_ap.rearrange("(b p) t -> b p t", p=P)          # [4, 128, 512]

    s1a = apool.tile([P, BLOCKS], FP, tag="s1a")    # sum dz*m2
    yta = apool.tile([P, BLOCKS], FP, tag="yta")    # sum y*(m2_0 - m2_10)
    for b in range(BLOCKS):
        tg32 = spool.tile([P, T], FP, tag="tg32")
        nc.sync.dma_start(tg32[:], tgt_r[b]).annotate(f"tdma{b}")
        q32 = qpool.tile([P, T * D], FP, tag="q32")
        nc.sync.dma_start(q32[:], inp_r[b]).annotate(f"qdma{b}")

        q3 = q32[:].rearrange("p (t i) -> p t i", i=D)
        yb = tg32[:].unsqueeze(2).broadcast_to([P, T, D])

        # m2 = (j - 10*[q >= y])^2   (fp16 exact ints; fused custom op)
        m2 = mpool.tile([P, T * D], F16, tag="m2")
        m3 = m2[:].rearrange("p (t i) -> p t i", i=D)
        nc.vector._custom_dve(
            CRPS_SEL_SQ2, out=m3, in0=q3, in1=yb, s0=10.0, s1=float(D),
        ).annotate(f"m2_{b}")

        # dz: interior on Pool, edge columns on DVE (f32 in -> f16 out)
        dz = mpool.tile([P, T * D], F16, tag="dz")
        dz3 = dz[:].rearrange("p (t i) -> p t i", i=D)
        nc.gpsimd.tensor_tensor(
            dz3[:, :, 1:10], q3[:, :, 2:11], q3[:, :, 0:9], AluOpType.subtract
        ).annotate(f"dz{b}")
        nc.vector.scalar_tensor_tensor(
            dz3[:, :, 0:1], q3[:, :, 0:1], 1.0, q3[:, :, 1:2],
            AluOpType.mult, AluOpType.add,
        ).annotate(f"dze0_{b}")
        nc.vector.scalar_tensor_tensor(
            dz3[:, :, 10:11], q3[:, :, 9:10], -1.0, q3[:, :, 10:11],
            AluOpType.mult, AluOpType.subtract,
        ).annotate(f"dze1_{b}")

        # prod = dz*m2 in place on dz (front cols DVE, back cols Pool)
        s = PROD_SPLIT * T
        nc.vector.tensor_tensor(
            dz[:, :s], dz[:, :s], m2[:, :s], AluOpType.mult
        ).annotate(f"prodv{b}")
        nc.gpsimd.tensor_tensor(
            dz[:, s:], dz[:, s:], m2[:, s:], AluOpType.mult
        ).annotate(f"prodp{b}")

        # S = sum prod  (ACT accumulate; scratch out over dead m2... m2 cols
        # 0/10 still feed the y-term, so scratch goes to a small spool tile)
        scr = mpool.tile([P, T * D], F16, tag="scr")
        nc.scalar.activation(
            scr[:], dz[:], ACT.Copy, accum_out=s1a[:, b : b + 1]
        ).annotate(f"sacc{b}")

        # y-term: md = m2_0 - m2_10 ; yterm = sum y*md
        md = spool.tile([P, T], F16, tag="md")
        nc.vector.tensor_tensor(
            md[:], m3[:, :, 0:1].squeeze(2), m3[:, :, 10:11].squeeze(2),
            AluOpType.subtract,
        ).annotate(f"md{b}")
        nc.vector.scalar_tensor_tensor(
            md[:], tg32[:], 1.0, md[:], AluOpType.mult, AluOpType.mult,
            accum_out=yta[:, b : b + 1],
        ).annotate(f"yterm{b}")
    # out = (S/200 - yterm/100)/T for all blocks
    r1 = apool.tile([P, BLOCKS], FP, tag="r1")
    nc.vector.tensor_scalar_mul(r1[:], yta[:], 0.01)
    r2 = apool.tile([P, BLOCKS], FP, tag="r2")
    nc.vector.scalar_tensor_tensor(
        r2[:], s1a[:], 0.005, r1[:], AluOpType.mult, AluOpType.subtract
    )
    r3 = apool.tile([P, BLOCKS], FP, tag="r3")
    nc.vector.tensor_scalar_mul(r3[:], r2[:], 1.0 / T)
    nc.sync.dma_start(out_ap.rearrange("(b p) -> p b", p=P), r3[:])


def _build_nc(repeat=1):
    nc = bacc.Bacc("TRN2", target_bir_lowering=False, debug=False,
                   num_devices=N_CORES)
    inp = nc.dram_tensor("inp", [N_LOC, T, D], FP, kind="ExternalInput").ap()
    tgt = nc.dram_tensor("target", [N_LOC, T], FP, kind="ExternalInput").ap()
    out = nc.dram_tensor("out", [N_LOC], FP, kind="ExternalOutput").ap()
    unroll = next(u for u in (8, 4, 2, 1) if repeat % u == 0)
    with tile.TileContext(nc) as tc:
        with (
            tc.tile_pool(name="qp", bufs=3) as qpool,
            tc.tile_pool(name="mp", bufs=3) as mpool,
            tc.tile_pool(name="sp", bufs=3) as spool,
            tc.tile_pool(name="acc", bufs=2) as apool,
        ):
            pools = (qpool, mpool, spool, apool)
            if repeat == 1:
                build_crps_kernel(tc, out, inp, tgt, pools)
            else:
                with tc.For_i(0, repeat // unroll, 1, staggered_reset=True):
                    for _ in range(unroll):
                        build_crps_kernel(tc, out, inp, tgt, pools)
    nc.compile()
    return nc


_NC_CACHE = {}


def get_nc(repeat=1):
    if repeat not in _NC_CACHE:
        _NC_CACHE[repeat] = _build_nc(repeat)
    return _NC_CACHE[repeat]


def kernel(inp: np.ndarray, target: np.ndarray) -> np.ndarray:
    inp = np.ascontiguousarray(inp, dtype=np.float32)
    target = np.ascontiguousarray(target, dtype=np.float32)
    nc = get_nc()
    in_maps = [
        {
            "inp": inp[c * N_LOC : (c + 1) * N_LOC],
            "target": target[c * N_LOC : (c + 1) * N_LOC],
        }
        for c in range(N_CORES)
    ]
    res = run_bass_kernel_spmd(nc, in_maps, core_ids=list(range(N_CORES)))
    return np.concatenate([res.results[c]["out"] for c in range(N_CORES)])
